# revision 16
# baseline (speedup 1.0000x reference)
"""Trainium2 Bass kernel for an 8-expert top-2 MoE block (B=4, T=2048, C=1024, H=4C).

Strategy (expert-parallel, per the sharding hint):
  - Host computes the gate (logits -> top-2 -> softmax) and the data-dependent
    sharding: each of the 8 NeuronCores owns one expert and receives only the
    tokens routed to it (the "all-to-all dispatch", done host-side as the
    sharding step), transposed to [C, cap] and cast to bf16.
  - Each core runs gelu(x @ w1 + b1) @ w2 + b2 over its routed tokens in bf16
    with fp32 PSUM accumulation, keeping activations transposed ([feature,
    token] layout) so weights are always the stationary matmul operand and
    biases are per-partition scalars fused into ScalarE activations.
  - Host applies the top-2 combine weights and scatter-adds the per-expert
    outputs back into the full [B, T, C] tensor (the "combine"/unshard).
"""

import sys

for _p in ("/opt/trn_rl_repo", "/root/.axon_site/_ro/trn_rl_repo"):
    if _p not in sys.path:
        sys.path.insert(0, _p)

from contextlib import ExitStack

import ml_dtypes
import numpy as np

import bass_rust
import concourse.bass as bass
import concourse.mybir as mybir
import concourse.tile as tile
from concourse.bass import ts
from concourse.bass_utils import run_bass_kernel_spmd


def _legalize_waits(nc: "bass.Bass") -> None:
    """Split multi-wait instructions into standalone EventSemaphore waits.

    The walrus build here accepts at most one sync-wait command per
    instruction (setupSyncWait "Too many sync wait commands"), but Tile
    attaches every outstanding dependency to the consuming instruction.
    Hoist all but the last wait onto same-engine EventSemaphore
    instructions placed immediately before the consumer: the engine's
    sequencer processes them in order, so the dependency still holds.
    """

    def fix_block(bb):
        out = []
        for inst in bb.instructions:
            si = inst.sync_info
            if si is not None and len(si.on_wait) > 1:
                waits = list(si.on_wait)
                for k, w in enumerate(waits[:-1]):
                    ev = bass_rust.InstEventSemaphore(
                        name=f"{inst.name}-lw{k}", ins=[], outs=[],
                        engine=inst.engine,
                    )
                    ev.sync_info = bass_rust.SyncInfo(on_wait=[w], on_update=[])
                    out.append(ev)
                inst.sync_info = bass_rust.SyncInfo(
                    on_wait=[waits[-1]], on_update=list(si.on_update)
                )
            out.append(inst)
        bb.instructions = out
        for sub in getattr(bb, "blocks", []) or []:
            fix_block(sub)

    for fn in nc.m.functions:
        for bb in fn.blocks:
            fix_block(bb)

B, T, C, E = 4, 2048, 1024, 8
H = 4 * C
N = B * T
TOP_K = 2
P = 128            # partitions
N_CHUNKS = 7       # token chunks per core
CK = C // P        # 8 contraction chunks for x @ w1
HK = H // P        # 32 contraction chunks for h @ w2

BF16 = mybir.dt.bfloat16
F32 = mybir.dt.float32


def _build_nc(cap: int, TOK: int) -> bass.Bass:
    nc = bass.Bass()
    xT = nc.declare_dram_parameter("xT", [C, cap], BF16, isOutput=False)
    w1 = nc.declare_dram_parameter("w1", [C, H], BF16, isOutput=False)
    w2 = nc.declare_dram_parameter("w2", [H, C], BF16, isOutput=False)
    b1 = nc.declare_dram_parameter("b1", [H], F32, isOutput=False)
    b2 = nc.declare_dram_parameter("b2", [C], F32, isOutput=False)
    yT = nc.declare_dram_parameter("yT", [C, cap], F32, isOutput=True)

    n_chunks = cap // TOK
    gelu = mybir.ActivationFunctionType.Gelu
    ident = mybir.ActivationFunctionType.Identity

    with tile.TileContext(nc) as tc, ExitStack() as ctx:
        w1p = ctx.enter_context(tc.tile_pool(name="w1p", bufs=CK))
        w2p = ctx.enter_context(tc.tile_pool(name="w2p", bufs=HK))
        cst = ctx.enter_context(tc.tile_pool(name="cst", bufs=1))
        xp = ctx.enter_context(tc.tile_pool(name="xp", bufs=2))
        hp = ctx.enter_context(tc.tile_pool(name="hp", bufs=HK))
        op = ctx.enter_context(tc.tile_pool(name="op", bufs=4))
        psA = ctx.enter_context(tc.tile_pool(name="psA", bufs=6, space="PSUM"))
        psB = ctx.enter_context(tc.tile_pool(name="psB", bufs=2, space="PSUM"))

        def load_xt(t):
            xt = xp.tile([P, CK, TOK], BF16, tag="xt")
            for c in range(CK):
                nc.sync.dma_start(xt[:, c, :], xT[c * P:(c + 1) * P, ts(t, TOK)])
            return xt

        # Warm the PE HAM clock gate while the first DMAs stream: ~20
        # back-to-back dummy matmuls give the >=3.4us of sustained PE
        # activity that flips the clock from 1.2 to 2.4 GHz. The real
        # chunk-0 matmuls trickle in behind DMA waits and would otherwise
        # run cold until ~44us.
        dummy = cst.tile([P, 512], BF16, tag="dummy")
        nc.gpsimd.memset(dummy[:], 0.0)
        warm = psB.tile([P, 512], F32, tag="psB", name="warm")
        for i in range(20):
            nc.tensor.matmul(warm[:], dummy[:, :P], dummy[:, :512],
                             start=(i == 0), stop=(i == 19))

        # Emission order = DMA queue order: chunk-0 tokens first (small),
        # then the w1 stream that gates phase A, then biases (not needed
        # until the first gelu, ~40us in).
        xt0 = load_xt(0)
        w1_sb = []
        for c in range(CK):
            t_ = w1p.tile([P, H], BF16, tag="w1")
            nc.sync.dma_start(t_[:], w1[c * P:(c + 1) * P, :])
            w1_sb.append(t_)

        # Biases laid out with the feature dim on partitions: sb[p, i] = b[i*128 + p].
        b1_sb = cst.tile([P, HK], F32, tag="b1")
        nc.sync.dma_start(b1_sb[:], b1.rearrange("(a p) -> p a", p=P))
        b2_sb = cst.tile([P, CK], F32, tag="b2")
        nc.sync.dma_start(b2_sb[:], b2.rearrange("(a p) -> p a", p=P))
        # Absorb the bias DMA waits on ScalarE here: the activation
        # instruction's sync struct only fits one wait, and the first
        # gelu/identity already needs a PE wait.
        scr1 = cst.tile([P, 1], F32, tag="scr1")
        nc.scalar.copy(scr1[:], b1_sb[:, 0:1])
        scr2 = cst.tile([P, 1], F32, tag="scr2")
        nc.scalar.copy(scr2[:], b2_sb[:, 0:1])

        def phase_a(xt, c_outer_blocks=False):
            """hT[h] = gelu(w1[:, h].T @ xT + b1[h]) for all HK chunks.

            c_outer_blocks: issue the c-contraction outermost over blocks
            of 6 PSUM banks, so chunk 0's matmuls start as soon as w1[c]
            lands instead of stalling until the whole w1 is resident.
            """
            hts = [None] * HK
            blk = 6 if c_outer_blocks else 1
            for hb0 in range(0, HK, blk):
                hs = list(range(hb0, min(hb0 + blk, HK)))
                pas = [psA.tile([P, TOK], F32, tag="psA", name=f"pa{h}")
                       for h in hs]
                for c in range(CK):
                    for j, h in enumerate(hs):
                        nc.tensor.matmul(
                            pas[j][:],
                            w1_sb[c][:, h * P:(h + 1) * P],
                            xt[:, c, :],
                            start=(c == 0),
                            stop=(c == CK - 1),
                        )
                for j, h in enumerate(hs):
                    ht = hp.tile([P, TOK], BF16, tag="ht")
                    nc.scalar.activation(ht[:], pas[j][:], gelu,
                                         bias=b1_sb[:, h:h + 1])
                    hts[h] = ht
            return hts

        def phase_b(t, hts):
            """yT[c] = w2[:, c].T @ hT + b2[c]; DMA out."""
            for c in range(CK):
                pb = psB.tile([P, TOK], F32, tag="psB")
                for h in range(HK):
                    nc.tensor.matmul(
                        pb[:],
                        w2_sb[h][:, c * P:(c + 1) * P],
                        hts[h][:],
                        start=(h == 0),
                        stop=(h == HK - 1),
                    )
                ot = op.tile([P, TOK], F32, tag="ot")
                nc.scalar.activation(ot[:], pb[:], ident, bias=b2_sb[:, c:c + 1])
                nc.sync.dma_start(yT[c * P:(c + 1) * P, ts(t, TOK)], ot[:])

        # Chunk 0 phase A is emitted before the w2 loads so its DMAs and
        # matmuls aren't queued behind 8MB of w2 traffic.
        hts0 = phase_a(xt0, c_outer_blocks=True)
        w2_sb = []
        for h in range(HK):
            t_ = w2p.tile([P, C], BF16, tag="w2")
            nc.sync.dma_start(t_[:], w2[h * P:(h + 1) * P, :])
            w2_sb.append(t_)
        phase_b(0, hts0)

        for t in range(1, n_chunks):
            xt = load_xt(t)
            hts = phase_a(xt)
            phase_b(t, hts)

    _legalize_waits(nc)
    return nc


_NC_CACHE: dict[int, bass.Bass] = {}
_LAST_IN_MAPS: list | None = None
_LAST_RESULTS = None


def _routing(xf: np.ndarray, w_gate: np.ndarray):
    logits = xf.astype(np.float64) @ w_gate.astype(np.float64)       # [N, E]
    top_idx = np.argsort(-logits, axis=-1, kind="stable")[:, :TOP_K]  # [N, K]
    top_vals = np.take_along_axis(logits, top_idx, axis=-1)
    ex = np.exp(top_vals - top_vals.max(axis=-1, keepdims=True))
    scores = ex / ex.sum(axis=-1, keepdims=True)                      # [N, K]
    return top_idx, scores


def kernel(x, w_gate, w1, b1, w2, b2):
    x = np.asarray(x, dtype=np.float32)
    w_gate = np.asarray(w_gate, dtype=np.float32)
    w1 = np.asarray(w1, dtype=np.float32)
    b1 = np.asarray(b1, dtype=np.float32)
    w2 = np.asarray(w2, dtype=np.float32)
    b2 = np.asarray(b2, dtype=np.float32)

    xf = x.reshape(N, C)
    top_idx, scores = _routing(xf, w_gate)

    idx, cw = [], []
    for e in range(E):
        hit = top_idx == e                       # [N, K]
        tok = np.nonzero(hit.any(axis=-1))[0]
        idx.append(tok)
        cw.append((scores * hit).sum(axis=-1)[tok].astype(np.float32))

    # Token chunk: split the max per-expert count into N_CHUNKS chunks,
    # rounded up to a multiple of 16 (<=512 so one PSUM bank holds a tile).
    maxc = max(max(len(i) for i in idx), 16)
    tokc = min(512, -(-(-(-maxc // N_CHUNKS)) // 16) * 16)
    cap = -(-maxc // tokc) * tokc

    nc = _NC_CACHE.get((cap, tokc))
    if nc is None:
        nc = _NC_CACHE[(cap, tokc)] = _build_nc(cap, tokc)

    in_maps = []
    for e in range(E):
        xTe = np.zeros((C, cap), dtype=ml_dtypes.bfloat16)
        xTe[:, :len(idx[e])] = xf[idx[e]].T.astype(ml_dtypes.bfloat16)
        in_maps.append({
            "xT": xTe,
            "w1": w1[e].astype(ml_dtypes.bfloat16),
            "w2": w2[e].astype(ml_dtypes.bfloat16),
            "b1": b1[e],
            "b2": b2[e],
        })

    global _LAST_IN_MAPS, _LAST_RESULTS
    _LAST_IN_MAPS = in_maps
    res = run_bass_kernel_spmd(nc, in_maps, list(range(E)))
    _LAST_RESULTS = res

    out = np.zeros((N, C), dtype=np.float32)
    for e in range(E):
        ye = res.results[e]["yT"].T[:len(idx[e])]    # [n_e, C]
        out[idx[e]] += cw[e][:, None] * ye
    return out.reshape(B, T, C)


# revision 21
# speedup vs baseline: 1.0009x; 1.0009x over previous
"""Trainium2 Bass kernel for an 8-expert top-2 MoE block (B=4, T=2048, C=1024, H=4C).

Strategy (expert-parallel, per the sharding hint):
  - Host computes the gate (logits -> top-2 -> softmax) and the data-dependent
    sharding: each of the 8 NeuronCores owns one expert and receives only the
    tokens routed to it (the "all-to-all dispatch", done host-side as the
    sharding step), transposed to [C, cap] and cast to bf16.
  - Each core runs gelu(x @ w1 + b1) @ w2 + b2 over its routed tokens in bf16
    with fp32 PSUM accumulation, keeping activations transposed ([feature,
    token] layout) so weights are always the stationary matmul operand and
    biases are per-partition scalars fused into ScalarE activations.
  - Host applies the top-2 combine weights and scatter-adds the per-expert
    outputs back into the full [B, T, C] tensor (the "combine"/unshard).
"""

import sys

for _p in ("/opt/trn_rl_repo", "/root/.axon_site/_ro/trn_rl_repo"):
    if _p not in sys.path:
        sys.path.insert(0, _p)

from contextlib import ExitStack

import ml_dtypes
import numpy as np

import bass_rust
import concourse.bass as bass
import concourse.mybir as mybir
import concourse.tile as tile
from concourse.bass import ts
from concourse.bass_utils import run_bass_kernel_spmd


def _legalize_waits(nc: "bass.Bass") -> None:
    """Split multi-wait instructions into standalone EventSemaphore waits.

    The walrus build here accepts at most one sync-wait command per
    instruction (setupSyncWait "Too many sync wait commands"), but Tile
    attaches every outstanding dependency to the consuming instruction.
    Hoist all but the last wait onto same-engine EventSemaphore
    instructions placed immediately before the consumer: the engine's
    sequencer processes them in order, so the dependency still holds.
    """

    def fix_block(bb):
        out = []
        for inst in bb.instructions:
            si = inst.sync_info
            if si is not None and len(si.on_wait) > 1:
                waits = list(si.on_wait)
                for k, w in enumerate(waits[:-1]):
                    ev = bass_rust.InstEventSemaphore(
                        name=f"{inst.name}-lw{k}", ins=[], outs=[],
                        engine=inst.engine,
                    )
                    ev.sync_info = bass_rust.SyncInfo(on_wait=[w], on_update=[])
                    out.append(ev)
                inst.sync_info = bass_rust.SyncInfo(
                    on_wait=[waits[-1]], on_update=list(si.on_update)
                )
            out.append(inst)
        bb.instructions = out
        for sub in getattr(bb, "blocks", []) or []:
            fix_block(sub)

    for fn in nc.m.functions:
        for bb in fn.blocks:
            fix_block(bb)

B, T, C, E = 4, 2048, 1024, 8
H = 4 * C
N = B * T
TOP_K = 2
P = 128            # partitions
N_CHUNKS = 7       # token chunks per core
CK = C // P        # 8 contraction chunks for x @ w1
HK = H // P        # 32 contraction chunks for h @ w2

BF16 = mybir.dt.bfloat16
F32 = mybir.dt.float32


def _build_nc(cap: int, TOK: int) -> bass.Bass:
    nc = bass.Bass()
    xT = nc.declare_dram_parameter("xT", [C, cap], BF16, isOutput=False)
    w1 = nc.declare_dram_parameter("w1", [C, H], BF16, isOutput=False)
    w2 = nc.declare_dram_parameter("w2", [H, C], BF16, isOutput=False)
    b1 = nc.declare_dram_parameter("b1", [H], F32, isOutput=False)
    b2 = nc.declare_dram_parameter("b2", [C], F32, isOutput=False)
    yT = nc.declare_dram_parameter("yT", [C, cap], F32, isOutput=True)

    n_chunks = cap // TOK
    gelu = mybir.ActivationFunctionType.Gelu
    ident = mybir.ActivationFunctionType.Identity

    with tile.TileContext(nc) as tc, ExitStack() as ctx:
        w1p = ctx.enter_context(tc.tile_pool(name="w1p", bufs=4 * CK))
        w2p = ctx.enter_context(tc.tile_pool(name="w2p", bufs=HK))
        cst = ctx.enter_context(tc.tile_pool(name="cst", bufs=1))
        xp = ctx.enter_context(tc.tile_pool(name="xp", bufs=2))
        hp = ctx.enter_context(tc.tile_pool(name="hp", bufs=HK))
        op = ctx.enter_context(tc.tile_pool(name="op", bufs=4))
        psA = ctx.enter_context(tc.tile_pool(name="psA", bufs=6, space="PSUM"))
        psB = ctx.enter_context(tc.tile_pool(name="psB", bufs=2, space="PSUM"))

        def load_xt(t):
            xt = xp.tile([P, CK, TOK], BF16, tag="xt")
            for c in range(CK):
                nc.sync.dma_start(xt[:, c, :], xT[c * P:(c + 1) * P, ts(t, TOK)])
            return xt

        # Warm the PE HAM clock gate while the first DMAs stream: ~20
        # back-to-back dummy matmuls give the >=3.4us of sustained PE
        # activity that flips the clock from 1.2 to 2.4 GHz. The real
        # chunk-0 matmuls trickle in behind DMA waits and would otherwise
        # run cold until ~44us.
        dummy = cst.tile([P, 512], BF16, tag="dummy")
        nc.gpsimd.memset(dummy[:], 0.0)
        warm = psB.tile([P, 512], F32, tag="psB", name="warm")
        for i in range(24):
            nc.tensor.matmul(warm[:], dummy[:, :P], dummy[:, :512],
                             start=(i == 0), stop=(i == 23))

        # Emission order = DMA queue order: chunk-0 tokens first (small),
        # then the w1 stream that gates phase A, then biases (not needed
        # until the first gelu, ~40us in).
        #
        # w1 is loaded as 4 H-slices x 8 C-blocks of [128, HS] each, slice
        # order matching phase A's h-group order: every 2MB slice completes
        # 4 h-groups' worth of weights, so PE reaches ~75% duty during the
        # serial ~22us w1 stream instead of stalling for the whole 8MB.
        HS = H // 4
        xt0 = load_xt(0)
        w1_sb = [[None] * 4 for _ in range(CK)]
        for hs in range(4):
            for c in range(CK):
                t_ = w1p.tile([P, HS], BF16, tag="w1", name=f"w1_{c}_{hs}")
                nc.sync.dma_start(t_[:], w1[c * P:(c + 1) * P, ts(hs, HS)])
                w1_sb[c][hs] = t_

        # Biases laid out with the feature dim on partitions: sb[p, i] = b[i*128 + p].
        b1_sb = cst.tile([P, HK], F32, tag="b1")
        nc.sync.dma_start(b1_sb[:], b1.rearrange("(a p) -> p a", p=P))
        b2_sb = cst.tile([P, CK], F32, tag="b2")
        nc.sync.dma_start(b2_sb[:], b2.rearrange("(a p) -> p a", p=P))
        # Absorb the bias DMA waits on ScalarE here: the activation
        # instruction's sync struct only fits one wait, and the first
        # gelu/identity already needs a PE wait.
        scr1 = cst.tile([P, 1], F32, tag="scr1")
        nc.scalar.copy(scr1[:], b1_sb[:, 0:1])
        scr2 = cst.tile([P, 1], F32, tag="scr2")
        nc.scalar.copy(scr2[:], b2_sb[:, 0:1])

        HPS = HK // 4  # h-groups per w1 slice

        def phase_a(xt):
            """hT[h] = gelu(w1[:, h].T @ xT + b1[h]) for all HK chunks.

            Blocks of 3 PSUM banks (psA bufs=6) so PE fills one triple
            while ScalarE drains the previous one.
            """
            hts = [None] * HK
            for hb0 in range(0, HK, 3):
                hs_grp = list(range(hb0, min(hb0 + 3, HK)))
                pas = [psA.tile([P, TOK], F32, tag="psA", name=f"pa{h}")
                       for h in hs_grp]
                for j, h in enumerate(hs_grp):
                    sl, off = divmod(h, HPS)
                    for c in range(CK):
                        nc.tensor.matmul(
                            pas[j][:],
                            w1_sb[c][sl][:, off * P:(off + 1) * P],
                            xt[:, c, :],
                            start=(c == 0),
                            stop=(c == CK - 1),
                        )
                for j, h in enumerate(hs_grp):
                    ht = hp.tile([P, TOK], BF16, tag="ht")
                    nc.scalar.activation(ht[:], pas[j][:], gelu,
                                         bias=b1_sb[:, h:h + 1])
                    hts[h] = ht
            return hts

        def phase_b(t, hts):
            """yT[c] = w2[:, c].T @ hT + b2[c]; DMA out."""
            for c in range(CK):
                pb = psB.tile([P, TOK], F32, tag="psB")
                for h in range(HK):
                    nc.tensor.matmul(
                        pb[:],
                        w2_sb[h][:, c * P:(c + 1) * P],
                        hts[h][:],
                        start=(h == 0),
                        stop=(h == HK - 1),
                    )
                ot = op.tile([P, TOK], F32, tag="ot")
                nc.scalar.activation(ot[:], pb[:], ident, bias=b2_sb[:, c:c + 1])
                nc.sync.dma_start(yT[c * P:(c + 1) * P, ts(t, TOK)], ot[:])

        # Chunk 0 phase A is emitted before the w2 loads so its DMAs and
        # matmuls aren't queued behind 8MB of w2 traffic.
        hts0 = phase_a(xt0)
        w2_sb = []
        for h in range(HK):
            t_ = w2p.tile([P, C], BF16, tag="w2")
            nc.sync.dma_start(t_[:], w2[h * P:(h + 1) * P, :])
            w2_sb.append(t_)
        phase_b(0, hts0)

        for t in range(1, n_chunks):
            xt = load_xt(t)
            hts = phase_a(xt)
            phase_b(t, hts)

    _legalize_waits(nc)
    return nc


_NC_CACHE: dict[int, bass.Bass] = {}
_LAST_IN_MAPS: list | None = None
_LAST_RESULTS = None


def _routing(xf: np.ndarray, w_gate: np.ndarray):
    logits = xf.astype(np.float64) @ w_gate.astype(np.float64)       # [N, E]
    top_idx = np.argsort(-logits, axis=-1, kind="stable")[:, :TOP_K]  # [N, K]
    top_vals = np.take_along_axis(logits, top_idx, axis=-1)
    ex = np.exp(top_vals - top_vals.max(axis=-1, keepdims=True))
    scores = ex / ex.sum(axis=-1, keepdims=True)                      # [N, K]
    return top_idx, scores


def kernel(x, w_gate, w1, b1, w2, b2):
    x = np.asarray(x, dtype=np.float32)
    w_gate = np.asarray(w_gate, dtype=np.float32)
    w1 = np.asarray(w1, dtype=np.float32)
    b1 = np.asarray(b1, dtype=np.float32)
    w2 = np.asarray(w2, dtype=np.float32)
    b2 = np.asarray(b2, dtype=np.float32)

    xf = x.reshape(N, C)
    top_idx, scores = _routing(xf, w_gate)

    idx, cw = [], []
    for e in range(E):
        hit = top_idx == e                       # [N, K]
        tok = np.nonzero(hit.any(axis=-1))[0]
        idx.append(tok)
        cw.append((scores * hit).sum(axis=-1)[tok].astype(np.float32))

    # Token chunk: split the max per-expert count into N_CHUNKS chunks,
    # rounded up to a multiple of 16 (<=512 so one PSUM bank holds a tile).
    maxc = max(max(len(i) for i in idx), 16)
    tokc = min(512, -(-(-(-maxc // N_CHUNKS)) // 16) * 16)
    cap = -(-maxc // tokc) * tokc

    nc = _NC_CACHE.get((cap, tokc))
    if nc is None:
        nc = _NC_CACHE[(cap, tokc)] = _build_nc(cap, tokc)

    in_maps = []
    for e in range(E):
        xTe = np.zeros((C, cap), dtype=ml_dtypes.bfloat16)
        xTe[:, :len(idx[e])] = xf[idx[e]].T.astype(ml_dtypes.bfloat16)
        in_maps.append({
            "xT": xTe,
            "w1": w1[e].astype(ml_dtypes.bfloat16),
            "w2": w2[e].astype(ml_dtypes.bfloat16),
            "b1": b1[e],
            "b2": b2[e],
        })

    global _LAST_IN_MAPS, _LAST_RESULTS
    _LAST_IN_MAPS = in_maps
    res = run_bass_kernel_spmd(nc, in_maps, list(range(E)))
    _LAST_RESULTS = res

    out = np.zeros((N, C), dtype=np.float32)
    for e in range(E):
        ye = res.results[e]["yT"].T[:len(idx[e])]    # [n_e, C]
        out[idx[e]] += cw[e][:, None] * ye
    return out.reshape(B, T, C)
